# revision 1
# baseline (speedup 1.0000x reference)
"""Causal multi-head self-attention on 8 TRN2 NeuronCores.

Sharding: batch (2) x head-groups (4) -> 8 cores. Each core computes the
qkv projection for its 4 heads of its batch, full causal attention for
those heads, and a partial output projection (its head slice of w_out);
the host sums the 4 partials per batch.

Per-core pipeline (matmuls in float32r: 13-bit-mantissa fp32, 1 cyc/row):
  A) x -> x^T via PE transposes; Q^T,K^T (head dims on partitions) and
     V natural (with a ones column appended per head) via matmuls
     against host-pre-transposed weights.
  B) per (head, 512-wide q tile): S^T = K^T.T @ Q^T (k on partitions),
     P^T = exp(S^T/8) via ACT, staircase causal mask on diagonal
     blocks, O^T += [V|1].T @ P^T accumulated in PSUM — the ones column
     of V makes row 64 the softmax denominator. Normalize with DVE
     reciprocal + PE broadcast of 1/denom.
  C) partial[s, :] = sum_h aoT_h.T @ woT_h -> DRAM.
"""

import math
import numpy as np

import concourse.bacc as bacc
import concourse.mybir as mybir
import concourse.tile as tile
from concourse.masks import make_identity
from concourse.bass_utils import run_bass_kernel_spmd

F32 = mybir.dt.float32
F32R = mybir.dt.float32r
EXP = mybir.ActivationFunctionType.Exp

D_MODEL = 1024
HEAD_DIM = 64
B, S = 2, 2048
N_CORES = 8
OLOC = 256                  # 4 heads x 64 dims per core
SCALE = 1.0 / math.sqrt(HEAD_DIM)

QT = 512                    # q tile (free dim of S^T / O^T)
NQT = S // QT
KB = 128                    # k block (partitions of S^T)
SB = 256                    # s tile in projection phase A

_CACHE = {}


def build_nc():
    nc = bacc.Bacc("TRN2", target_bir_lowering=False, debug=False)

    x_d = nc.dram_tensor("x", [S, D_MODEL], F32, kind="ExternalInput")
    wqk_d = nc.dram_tensor("wqk_t", [D_MODEL, 512], F32R, kind="ExternalInput")
    wv_d = nc.dram_tensor("wv_t", [D_MODEL, OLOC], F32R, kind="ExternalInput")
    wo_d = nc.dram_tensor("wo_t", [OLOC, D_MODEL], F32R, kind="ExternalInput")
    out_d = nc.dram_tensor("out", [S, D_MODEL], F32, kind="ExternalOutput")

    with tile.TileContext(nc) as tc:
        with (
            tc.tile_pool(name="persist", bufs=1) as pp,
            tc.tile_pool(name="work", bufs=2) as wp,
            tc.tile_pool(name="psum", bufs=1, space="PSUM") as psp,
        ):
            ident = pp.tile([128, 128], F32)
            make_identity(nc, ident[:])

            # staircase causal mask: M[p, c] = 1 if p <= c - 384 else 0.
            # Slice [:, (3-j)*128 :][:512] masks diagonal subblock j.
            mask = pp.tile([128, 896], F32)
            nc.gpsimd.memset(mask[:], 1.0)
            nc.gpsimd.affine_select(
                out=mask[:], in_=mask[:],
                compare_op=mybir.AluOpType.is_ge,
                fill=0.0, base=-384,
                pattern=[[1, 896]], channel_multiplier=-1,
            )

            ones_f = pp.tile([1, 64], F32)
            nc.gpsimd.memset(ones_f[:], 1.0)
            ones_r = pp.tile([1, 64], F32R)
            nc.vector.tensor_copy(ones_r[:], ones_f[:])
            ones4 = pp.tile([128, 4, 1], F32)
            nc.gpsimd.memset(ones4[:], 1.0)

            # weights (pre-transposed on host)
            wqk = [pp.tile([128, 512], F32R, name=f"wqk{i}") for i in range(8)]
            wv = [pp.tile([128, OLOC], F32R, name=f"wv{i}") for i in range(8)]
            for i in range(8):
                nc.sync.dma_start(wqk[i][:], wqk_d[i * 128:(i + 1) * 128, :])
                nc.sync.dma_start(wv[i][:], wv_d[i * 128:(i + 1) * 128, :])
            wo = [pp.tile([64, D_MODEL], F32R, name=f"wo{h}") for h in range(4)]
            for h in range(4):
                nc.sync.dma_start(wo[h][:], wo_d[h * 64:(h + 1) * 64, :])

            # persistent activations
            qkT = [pp.tile([128, S], F32R, name=f"qkT{ob}") for ob in range(4)]
            v_sb = [pp.tile([128, 4 * 65], F32R, name=f"v{j}")
                    for j in range(S // 128)]
            aoT = [pp.tile([64, S], F32R, name=f"aoT{h}") for h in range(4)]

            # ---- Phase A: x^T, Q^T/K^T, V ----
            for sb in range(S // SB):
                xn = wp.tile([128, 2, D_MODEL], F32, tag="xn", bufs=2)
                for j in range(2):
                    nc.sync.dma_start(
                        xn[:, j, :],
                        x_d[sb * SB + j * 128:sb * SB + (j + 1) * 128, :])
                xT = wp.tile([128, 8, SB], F32R, tag="xT", bufs=2)
                for it in range(8):
                    pt = psp.tile([128, SB], F32, tag="mm", bufs=2)
                    for j in range(2):
                        nc.tensor.matmul(
                            pt[:, j * 128:(j + 1) * 128],
                            xn[:, j, it * 128:(it + 1) * 128],
                            ident[:], is_transpose=True,
                            start=True, stop=True)
                    nc.vector.tensor_copy(xT[:, it, :], pt[:])
                # Q^T / K^T: psum (128 o, SB s) accumulated over 8 i-tiles
                for ob in range(4):
                    pqk = psp.tile([128, SB], F32, tag="mm", bufs=2)
                    for it in range(8):
                        nc.tensor.matmul(
                            pqk[:],
                            wqk[it][:, ob * 128:(ob + 1) * 128],
                            xT[:, it, :],
                            start=(it == 0), stop=(it == 7))
                    nc.vector.tensor_copy(qkT[ob][:, sb * SB:(sb + 1) * SB], pqk[:])
                # V natural per 128-row s block, interleaved [V_h | 1]
                for j in range(2):
                    pv = psp.tile([128, OLOC], F32, tag="mm", bufs=2)
                    for it in range(8):
                        nc.tensor.matmul(
                            pv[:],
                            xT[:, it, j * 128:(j + 1) * 128],
                            wv[it][:],
                            start=(it == 0), stop=(it == 7))
                    vt = v_sb[sb * 2 + j]
                    vt3 = vt.rearrange("p (h d) -> p h d", h=4)
                    nc.vector.tensor_copy(vt3[:, :, 64:65], ones4[:])
                    nc.vector.tensor_copy(
                        vt3[:, :, 0:64],
                        pv[:].rearrange("p (h d) -> p h d", h=4))

            # ---- Phase B: attention ----
            for h in range(4):
                q_ap = qkT[h // 2][64 * (h % 2):64 * (h % 2) + 64, :]
                k_ap = qkT[2 + h // 2][64 * (h % 2):64 * (h % 2) + 64, :]
                for qt in range(NQT):
                    po = psp.tile([65, QT], F32, tag="po", bufs=2)
                    nkb = (qt + 1) * (QT // KB)   # 4, 8, 12, 16
                    for grp in range(nkb // 2):
                        pst = psp.tile([128, 1024], F32, tag="pst", bufs=1)
                        for u in range(2):
                            kb = grp * 2 + u
                            nc.tensor.matmul(
                                pst[:, u * 512:(u + 1) * 512],
                                k_ap[:, kb * KB:(kb + 1) * KB],
                                q_ap[:, qt * QT:(qt + 1) * QT],
                                start=True, stop=True)
                        p_t = wp.tile([128, 1024], F32R, tag="p_t", bufs=3)
                        nc.scalar.activation(p_t[:], pst[:], EXP, scale=SCALE)
                        for u in range(2):
                            kb = grp * 2 + u
                            j = kb - (nkb - 4)
                            if j >= 0:  # diagonal band: staircase mask
                                nc.vector.tensor_mul(
                                    p_t[:, u * 512:(u + 1) * 512],
                                    p_t[:, u * 512:(u + 1) * 512],
                                    mask[:, (3 - j) * 128:(3 - j) * 128 + 512])
                            nc.tensor.matmul(
                                po[:],
                                v_sb[kb][:, h * 65:(h + 1) * 65],
                                p_t[:, u * 512:(u + 1) * 512],
                                start=(kb == 0), stop=(kb == nkb - 1),
                                skip_group_check=True)
                    # normalize: 1/denom, broadcast via PE, multiply
                    with nc.allow_low_precision(reason="f32r recip"):
                        recip = wp.tile([1, QT], F32R, tag="recip", bufs=2)
                        nc.vector.reciprocal(recip[:], po[64:65, :])
                    pbc = psp.tile([64, QT], F32, tag="pbc", bufs=1)
                    nc.tensor.matmul(pbc[:], ones_r[:], recip[:],
                                     start=True, stop=True)
                    rbc = wp.tile([64, QT], F32, tag="rbc", bufs=2)
                    nc.scalar.copy(rbc[:], pbc[:])
                    nc.vector.tensor_mul(
                        aoT[h][:, qt * QT:(qt + 1) * QT], po[0:64, :], rbc[:])

            # ---- Phase C: output projection (partial) ----
            for sb2 in range(S // 128):
                for ob in range(2):
                    pout = psp.tile([128, 512], F32, tag="mm", bufs=2)
                    for h in range(4):
                        nc.tensor.matmul(
                            pout[:],
                            aoT[h][:, sb2 * 128:(sb2 + 1) * 128],
                            wo[h][:, ob * 512:(ob + 1) * 512],
                            start=(h == 0), stop=(h == 3))
                    osb = wp.tile([128, 512], F32, tag="osb", bufs=3)
                    nc.vector.tensor_copy(osb[:], pout[:])
                    nc.sync.dma_start(
                        out_d[sb2 * 128:(sb2 + 1) * 128, ob * 512:(ob + 1) * 512],
                        osb[:])

    nc.compile()
    return nc


def make_in_maps(x, w_qkv, w_out):
    in_maps = []
    for c in range(N_CORES):
        b, g = divmod(c, 4)
        wq = w_qkv[g * OLOC:(g + 1) * OLOC, :]
        wk = w_qkv[D_MODEL + g * OLOC:D_MODEL + (g + 1) * OLOC, :]
        wv = w_qkv[2 * D_MODEL + g * OLOC:2 * D_MODEL + (g + 1) * OLOC, :]
        in_maps.append({
            "x": np.ascontiguousarray(x[b]),
            "wqk_t": np.ascontiguousarray(np.concatenate([wq, wk], axis=0).T),
            "wv_t": np.ascontiguousarray(wv.T),
            "wo_t": np.ascontiguousarray(w_out[:, g * OLOC:(g + 1) * OLOC].T),
        })
    return in_maps


def kernel(x, w_qkv, w_out):
    x = np.asarray(x, dtype=np.float32)
    w_qkv = np.asarray(w_qkv, dtype=np.float32)
    w_out = np.asarray(w_out, dtype=np.float32)

    if "nc" not in _CACHE:
        _CACHE["nc"] = build_nc()
    nc = _CACHE["nc"]

    in_maps = make_in_maps(x, w_qkv, w_out)
    _CACHE["in_maps"] = in_maps

    res = run_bass_kernel_spmd(nc, in_maps, list(range(N_CORES)))
    out = np.zeros((B, S, D_MODEL), dtype=np.float32)
    for c in range(N_CORES):
        out[c // 4] += res.results[c]["out"]
    return out

